# revision 32
# baseline (speedup 1.0000x reference)
"""Trainium2 Bass kernel for nn_Autoencoder__gen204 (8-core data parallel).

Network: enc(2048->128->64->32 relu MLP) -> 4-qubit statevector circuit on
latent[:, :4] -> dec(4->64->128->2048 relu MLP).

Key transform: the quantum circuit is RX-encoding (per-sample angles) followed
by a FIXED unitary V(qw). E_q = Tr(rho . V^dag Z_q V) with rho a product state
whose per-qubit Bloch vectors are (0, -sin t, cos t). Expanding the fixed
observable in the Pauli basis, only {I,Y,Z}^4 strings survive, so
refined = phi @ W81 where phi is the 81-dim tensor product of per-qubit
features [1, sin t, cos t] and W81 is an (81,4) matrix computed on host from
qw (Y-string signs folded in). W81 @ dec_w0 then folds into the decoder, so
the whole model is matmuls + relu + sin/cos + 3 elementwise products.

Device layout: feature-major (features on SBUF partitions, batch on the free
dim). The batch (2048 cols/core) is processed in 8 chunks of 256 columns.
The host pre-casts x to bf16 (identical numerics to an in-DMA cast, half the
HBM read) and packs it so each chunk is one contiguous DRAM block; every
load/store rides the single FIFO SWDGE queue in priority order (w0p, bia,
x0, other weights, x1..x7, outputs), so the DMA engines stream back-to-back
and outputs drain strictly after x.

PSUM discipline (8 banks total, bank-granular allocation): the mid-chain's
eight stages are packed pairwise into four [.., 512] banks (two 256-col
halves each) with per-stage tags and bufs=1, so consecutive chunks' chains
pipeline stage-wise (chunk n+1's stage-k write only waits on chunk n's
stage-k readers) instead of serializing through a shared rotation. L1 gets
1 bank, the decoder's pair-matmul groups 2, and 1 bank holds the ballast
target: dummy matmuls woven into the PE stream at stall points keep the HAM
clock gate at 2.4 GHz (any recurring idle drops the PE to 1.2 GHz).

The fp16 output is staged in SBUF and written one contiguous DMA per chunk;
the host unpacks back to (B, D) f32 (|out| <= ~1e-3, so fp16 keeps ~2.4e-4
relative precision).
"""

import ml_dtypes
import numpy as np

import concourse.bass as bass
import concourse.mybir as mybir
import concourse.tile as tile
from concourse import bacc
from concourse.bass_utils import run_bass_kernel_spmd

# ----- problem constants (hardcoded per contract) -----
B, D, H1, H2, L = 16384, 2048, 128, 64, 32
NQ, NL = 4, 3
NCORES = 8
BL = B // NCORES  # 2048 batch per core
P = 128
KD = D // P  # 16 k-chunks for the D contraction
F32 = mybir.dt.float32
BF16 = mybir.dt.bfloat16
F16 = mybir.dt.float16

CWS = (256,) * 8  # per-chunk batch widths
NCH = len(CWS)
CMAX = max(CWS)
C0S = tuple(sum(CWS[:i]) for i in range(NCH))   # batch col offset per chunk
XT = KD * BL                                    # flat packed cols (32768)
XOFF = tuple(KD * c0 for c0 in C0S)             # packed col offset per chunk

# =====================================================================
# Host-side quantum-circuit collapse: qw -> W81 (81, 4)
# =====================================================================

_I2 = np.eye(2, dtype=np.complex128)
_SY = np.array([[0, -1j], [1j, 0]], dtype=np.complex128)
_SZ = np.array([[1, 0], [0, -1]], dtype=np.complex128)
_CNOT4 = np.array(
    [[1, 0, 0, 0], [0, 1, 0, 0], [0, 0, 0, 1], [0, 0, 1, 0]], dtype=np.complex128
).reshape(2, 2, 2, 2)
_bits = (np.arange(2**NQ)[:, None] >> np.arange(NQ - 1, -1, -1)) & 1
_Z_SIGNS = (1 - 2 * _bits).astype(np.float64)  # (16, 4)


def _rot_mat(phi, theta, omega):
    ez = np.exp(-0.5j * phi)
    rz1 = np.array([[ez, 0], [0, np.conj(ez)]], dtype=np.complex128)
    c, s = np.cos(theta / 2), np.sin(theta / 2)
    ry = np.array([[c, -s], [s, c]], dtype=np.complex128)
    eo = np.exp(-0.5j * omega)
    rz2 = np.array([[eo, 0], [0, np.conj(eo)]], dtype=np.complex128)
    return rz2 @ ry @ rz1


def _apply1(state, U, wire):
    state = np.tensordot(U, state, axes=[[1], [wire]])
    return np.moveaxis(state, 0, wire)


def _apply_cnot(state, c, t):
    state = np.tensordot(_CNOT4, state, axes=[[2, 3], [c, t]])
    return np.moveaxis(state, [0, 1], [c, t])


def _w81_from_qw(qw):
    qw = np.asarray(qw, dtype=np.float64)
    V = np.eye(16, dtype=np.complex128).reshape(2, 2, 2, 2, 16)
    for layer in range(NL):
        for q in range(NQ):
            V = _apply1(V, _rot_mat(*qw[layer, q]), q)
        for q in range(NQ - 1):
            V = _apply_cnot(V, q, q + 1)
    V = V.reshape(16, 16)
    paulis = [_I2, _SY, _SZ]  # digit 0 -> I(1), 1 -> Y(sin), 2 -> Z(cos)
    W = np.zeros((81, NQ), dtype=np.float64)
    for q in range(NQ):
        O = V.conj().T @ (_Z_SIGNS[:, q][:, None] * V)
        for k in range(81):
            d = [(k // 27) % 3, (k // 9) % 3, (k // 3) % 3, k % 3]
            Pm = np.kron(
                np.kron(paulis[d[0]], paulis[d[1]]),
                np.kron(paulis[d[2]], paulis[d[3]]),
            )
            alpha = np.trace(O @ Pm) / 16.0
            n_y = sum(1 for x in d if x == 1)
            W[k, q] = alpha.real * ((-1) ** n_y)  # t(Y) = -sin; feature is +sin
    return W


def _selection_matrices():
    """(9, 324) f32; column block q*81..(q+1)*81 is Sel_q mapping the 9-row
    feature stack [1, sin t0..3, cos t0..3] to the 81 phi rows."""
    S = np.zeros((9, 4 * 81), dtype=np.float32)
    for q in range(NQ):
        for k in range(81):
            d = (k // (3 ** (3 - q))) % 3
            row = 0 if d == 0 else (1 + q if d == 1 else 5 + q)
            S[row, q * 81 + k] = 1.0
    return S


# =====================================================================
# Bass program (one core; identical across the 8 cores)
# =====================================================================

_PROGRAM_CACHE = {}


def _build_program(debug=False, dec2_bias_zero=False, warmup=10, dec_dve=5,
                   wbufs=4, bal=(0, 0, 0), balw=256, tail_bal=0):
    key = (debug, dec2_bias_zero, warmup, dec_dve, wbufs, bal, balw, tail_bal)
    if key in _PROGRAM_CACHE:
        return _PROGRAM_CACHE[key]

    nc = bacc.Bacc("TRN2", target_bir_lowering=False, debug=debug)

    xpk = nc.dram_tensor("xpk", [P, XT], BF16, kind="ExternalInput")
    w0p = nc.dram_tensor("w0p", [P, D], BF16, kind="ExternalInput")
    w1 = nc.dram_tensor("w1", [H1, H2], BF16, kind="ExternalInput")
    # w2a = [0 | enc_w2[:, :4] | enc_w2[:, :4]]: the L3 matmul directly
    # produces the 9-row pre-activation whose Sin (with bias
    # [pi/2, b2, b2 + pi/2]) is the feature stack [1, sin t, cos t].
    w2a = nc.dram_tensor("w2a", [H2, 9], BF16, kind="ExternalInput")
    selc = nc.dram_tensor("selc", [9, 324], BF16, kind="ExternalInput")
    wf = nc.dram_tensor("wf", [81, H2], BF16, kind="ExternalInput")
    wd1 = nc.dram_tensor("wd1", [H2, H1], BF16, kind="ExternalInput")
    wd2 = nc.dram_tensor("wd2", [H1, D], BF16, kind="ExternalInput")
    bia = nc.dram_tensor("bia", [P, 21], F32, kind="ExternalInput")
    # fp16 output, packed per chunk: col XOFF[n] + k*cw + c maps to
    # out[d = k*128 + p, b = C0S[n] + c].
    outp = nc.dram_tensor("outp", [P, XT], F16, kind="ExternalOutput")

    Relu = mybir.ActivationFunctionType.Relu
    Sin = mybir.ActivationFunctionType.Sin

    with tile.TileContext(nc) as tc:
        with (
            tc.tile_pool(name="const", bufs=1) as cpool,
            tc.tile_pool(name="xin", bufs=NCH) as xpool,
            tc.tile_pool(name="work", bufs=wbufs) as wpool,
            tc.tile_pool(name="stage", bufs=NCH) as spool,
            tc.tile_pool(name="ps1p", bufs=1, space="PSUM") as ps1pool,
            tc.tile_pool(name="psmid", bufs=1, space="PSUM") as psmpool,
            tc.tile_pool(name="psout", bufs=2, space="PSUM") as psopool,
            tc.tile_pool(name="psbal", bufs=1, space="PSUM") as dpool,
        ):
            # ---- ALL loads ride the single FIFO SWDGE queue, in priority
            # order: [w0p, bia] -> x(0) -> [other weights] -> x(1..).
            # (A separate HWDGE weights queue gets starved by the x stream
            # for ~30us, stalling the first L1 eviction on bia.) ----
            xts = [
                xpool.tile([P, KD * CWS[n]], BF16, name=f"x_{n}", tag=f"xt{CWS[n]}")
                for n in range(NCH)
            ]
            nc.gpsimd.dma_start(
                out=xts[0][:], in_=xpk[:, XOFF[0] : XOFF[0] + KD * CWS[0]]
            )
            w0p_sb = cpool.tile([P, D], BF16, name="w0p_sb", tag="w0p_sb")
            nc.gpsimd.dma_start(out=w0p_sb[:], in_=w0p[:])
            bia_sb = cpool.tile([P, 21], F32, name="bia_sb", tag="bia_sb")
            nc.gpsimd.dma_start(out=bia_sb[:], in_=bia[:])

            w1_sb = cpool.tile([H1, H2], BF16, name="w1_sb", tag="w1_sb")
            nc.gpsimd.dma_start(out=w1_sb[:], in_=w1[:])
            w2a_sb = cpool.tile([H2, 9], BF16, name="w2a_sb", tag="w2a_sb")
            nc.gpsimd.dma_start(out=w2a_sb[:], in_=w2a[:])
            selc_sb = cpool.tile([9, 324], BF16, name="selc_sb", tag="selc_sb")
            nc.gpsimd.dma_start(out=selc_sb[:], in_=selc[:])
            wf_sb = cpool.tile([81, H2], BF16, name="wf_sb", tag="wf_sb")
            nc.gpsimd.dma_start(out=wf_sb[:], in_=wf[:])
            wd1_sb = cpool.tile([H2, H1], BF16, name="wd1_sb", tag="wd1_sb")
            nc.gpsimd.dma_start(out=wd1_sb[:], in_=wd1[:])
            wd2_sb = cpool.tile([H1, D], BF16, name="wd2_sb", tag="wd2_sb")
            nc.gpsimd.dma_start(out=wd2_sb[:], in_=wd2[:])

            for n in range(1, NCH):
                nc.gpsimd.dma_start(
                    out=xts[n][:], in_=xpk[:, XOFF[n] : XOFF[n] + KD * CWS[n]]
                )

            # out staging: one SBUF tile per chunk, written by the dec
            # evictions, drained by a single contiguous DMA per chunk.
            osts = [
                spool.tile([P, KD * CWS[n]], F16, name=f"ost_{n}", tag=f"ost{CWS[n]}")
                for n in range(NCH)
            ]

            state = {}

            def emit_l1(n):
                """16 accumulating matmuls + relu eviction for batch chunk n."""
                cw = CWS[n]
                ps1 = ps1pool.tile([P, CMAX], F32, name=f"ps1_{n}", tag="ps1")
                h1 = wpool.tile([P, CMAX], BF16, name=f"h1_{n}", tag="h1")
                state[n] = h1
                steps = [ballast] * bal[0]  # filler while waiting on x(n)
                for k in range(KD):
                    def mm(k=k, ps1=ps1, n=n, cw=cw):
                        nc.tensor.matmul(
                            ps1[:, :cw],
                            w0p_sb[:, k * P : (k + 1) * P],
                            xts[n][:, k * cw : (k + 1) * cw],
                            start=(k == 0),
                            stop=(k == KD - 1),
                        )
                    steps.append(mm)
                def ev(ps1=ps1, h1=h1, cw=cw):
                    nc.scalar.activation(h1[:, :cw], ps1[:, :cw], Relu,
                                         bias=bia_sb[:, 0:1])
                steps.append(ev)
                return steps

            def emit_mid(n):
                """Mid layers for batch chunk n (produces h4). The 8 PSUM
                stages pack pairwise into 4 banks (256-col halves); per-stage
                tags with bufs=1 pipeline consecutive chunks stage-wise."""
                cw = CWS[n]
                h1 = state[n][:, :cw]
                steps = []

                def mmstep(fn):
                    steps.append(fn)
                    steps.extend([ballast] * bal[1])

                m1 = psmpool.tile([H2, 2 * CMAX], F32, name=f"m1_{n}", tag="m1")
                m2 = psmpool.tile([81, 2 * CMAX], F32, name=f"m2_{n}", tag="m2")
                m3 = psmpool.tile([81, 2 * CMAX], F32, name=f"m3_{n}", tag="m3")
                m4 = psmpool.tile([P, 2 * CMAX], F32, name=f"m4_{n}", tag="m4")
                ps2 = m1[:, 0:cw]
                ps3 = m1[0:9, CMAX : CMAX + cw]
                psA0 = m2[:, 0:cw]
                psA1 = m2[:, CMAX : CMAX + cw]
                psA2 = m3[:, 0:cw]
                psA3 = m3[:, CMAX : CMAX + cw]
                ps4 = m4[0:H2, 0:cw]
                ps5 = m4[:, CMAX : CMAX + cw]

                h2 = wpool.tile([H2, CMAX], BF16, name=f"h2_{n}", tag="h2")
                mst = wpool.tile([9, CMAX], BF16, name=f"mst_{n}", tag="mst")
                s1 = wpool.tile([81, CMAX], F32, name=f"s1_{n}", tag="s1")
                t01 = wpool.tile([81, CMAX], F32, name=f"t01_{n}", tag="t01")
                s3 = wpool.tile([81, CMAX], F32, name=f"s3_{n}", tag="s3")
                t23 = wpool.tile([81, CMAX], F32, name=f"t23_{n}", tag="t23")
                phi = wpool.tile([81, CMAX], BF16, name=f"phi_{n}", tag="phi")
                h3 = wpool.tile([H2, CMAX], BF16, name=f"h3_{n}", tag="h3")
                h4 = wpool.tile([H1, CMAX], BF16, name=f"h4_{n}", tag="h4")

                mmstep(lambda: nc.tensor.matmul(ps2, w1_sb[:], h1, start=True, stop=True))
                steps.append(lambda: nc.scalar.activation(h2[:, :cw], ps2, Relu, bias=bia_sb[0:H2, 1:2]))
                mmstep(lambda: nc.tensor.matmul(ps3, w2a_sb[:], h2[:, :cw], start=True, stop=True))
                steps.append(lambda: nc.scalar.activation(mst[0:9, :cw], ps3, Sin, bias=bia_sb[0:9, 2:3]))
                mmstep(lambda: nc.tensor.matmul(psA0, selc_sb[:, 0:81], mst[0:9, :cw], start=True, stop=True))
                mmstep(lambda: nc.tensor.matmul(psA1, selc_sb[:, 81:162], mst[0:9, :cw], start=True, stop=True))
                steps.append(lambda: nc.vector.tensor_copy(s1[:, :cw], psA1))
                steps.append(lambda: nc.vector.tensor_mul(t01[:, :cw], psA0, s1[:, :cw]))
                mmstep(lambda: nc.tensor.matmul(psA2, selc_sb[:, 162:243], mst[0:9, :cw], start=True, stop=True))
                mmstep(lambda: nc.tensor.matmul(psA3, selc_sb[:, 243:324], mst[0:9, :cw], start=True, stop=True))
                steps.append(lambda: nc.scalar.copy(s3[:, :cw], psA3))
                steps.append(lambda: nc.vector.tensor_mul(t23[:, :cw], psA2, s3[:, :cw]))
                steps.append(lambda: nc.vector.tensor_mul(phi[:, :cw], t01[:, :cw], t23[:, :cw]))
                mmstep(lambda: nc.tensor.matmul(ps4, wf_sb[:], phi[:, :cw], start=True, stop=True))
                steps.append(lambda: nc.scalar.activation(h3[:, :cw], ps4, Relu, bias=bia_sb[0:H2, 3:4]))
                mmstep(lambda: nc.tensor.matmul(ps5, wd1_sb[:], h3[:, :cw], start=True, stop=True))
                steps.append(lambda: nc.scalar.activation(h4[:, :cw], ps5, Relu, bias=bia_sb[0:H1, 4:5]))
                state[("h4", n)] = h4
                return steps

            def emit_dec(n):
                """Decoder head for batch chunk n: 8 pair-matmul groups into
                [128, 2cw] PSUM banks, one wide eviction each into the chunk
                staging tile; then drain the chunk with one DMA."""
                cw = CWS[n]
                h4 = state[("h4", n)][:, :cw]
                ost = osts[n]
                steps = []
                for g in range(KD // 2):
                    mg = 2 * g
                    ps6 = psopool.tile([P, 2 * CMAX], F32, name=f"ps6_{n}_{g}", tag="pso")
                    for j in range(2):
                        def mm6(ps6=ps6, mg=mg, j=j, h4=h4, cw=cw):
                            nc.tensor.matmul(
                                ps6[:, j * cw : (j + 1) * cw],
                                wd2_sb[:, (mg + j) * P : (mg + j + 1) * P],
                                h4,
                                start=True,
                                stop=True,
                            )
                        steps.append(mm6)
                    steps.extend([ballast] * (bal[2] + (tail_bal if n >= NCH - 2 else 0)))
                    c0 = mg * cw
                    use_dve = (g + n) % 8 < dec_dve
                    if dec2_bias_zero:
                        if use_dve:
                            def ev6(ost=ost, ps6=ps6, c0=c0, cw=cw):
                                nc.vector.tensor_copy(ost[:, c0 : c0 + 2 * cw], ps6[:, : 2 * cw])
                        else:
                            def ev6(ost=ost, ps6=ps6, c0=c0, cw=cw):
                                nc.scalar.copy(ost[:, c0 : c0 + 2 * cw], ps6[:, : 2 * cw])
                        steps.append(ev6)
                    else:
                        for j in range(2):
                            if use_dve:
                                def ev6(ost=ost, ps6=ps6, mg=mg, j=j, c0=c0, cw=cw):
                                    nc.vector.tensor_scalar_add(
                                        ost[:, c0 + j * cw : c0 + (j + 1) * cw],
                                        ps6[:, j * cw : (j + 1) * cw],
                                        bia_sb[:, 5 + mg + j : 6 + mg + j],
                                    )
                            else:
                                def ev6(ost=ost, ps6=ps6, mg=mg, j=j, c0=c0, cw=cw):
                                    nc.scalar.add(
                                        ost[:, c0 + j * cw : c0 + (j + 1) * cw],
                                        ps6[:, j * cw : (j + 1) * cw],
                                        bia_sb[:, 5 + mg + j : 6 + mg + j],
                                    )
                            steps.append(ev6)
                # last chunk: drain in two halves so the final out-DMA's
                # bytes start moving before the last evictions finish
                w = KD * cw
                if n == NCH - 1:
                    def dma_out_a(n=n, ost=ost, w=w):
                        nc.gpsimd.dma_start(
                            out=outp[:, XOFF[n] : XOFF[n] + w // 2],
                            in_=ost[:, 0 : w // 2],
                        )
                    def dma_out_b(n=n, ost=ost, w=w):
                        nc.gpsimd.dma_start(
                            out=outp[:, XOFF[n] + w // 2 : XOFF[n] + w],
                            in_=ost[:, w // 2 : w],
                        )
                    steps.insert(3 * (len(steps) // 4), dma_out_a)
                    steps.append(dma_out_b)
                else:
                    def dma_out(n=n, ost=ost, w=w):
                        nc.gpsimd.dma_start(
                            out=outp[:, XOFF[n] : XOFF[n] + w], in_=ost[:]
                        )
                    steps.append(dma_out)
                return steps

            def ratio_merge(*streams):
                streams = [s for s in streams if s]
                out = []
                idx = [0] * len(streams)
                total = sum(len(s) for s in streams)
                for _ in range(total):
                    # advance the stream with the least relative progress
                    best, bestv = None, None
                    for si, s in enumerate(streams):
                        if idx[si] >= len(s):
                            continue
                        v = idx[si] / len(s)
                        if bestv is None or v < bestv:
                            best, bestv = si, v
                    out.append(streams[best][idx[best]])
                    idx[best] += 1
                return out

            # preload the Sin ACT table set while the Scalar engine is idle
            # (otherwise the first Sin swaps tables mid-kernel, serializing
            # ~2.6us of eviction work)
            dsin = cpool.tile([1, 8], F32, name="dsin", tag="dsin")
            nc.vector.memset(dsin[:], 0.0)
            nc.scalar.activation(dsin[:, 4:8], dsin[:, 0:4], Sin)

            # PE warm-up + ballast: dummy matmuls into a dedicated PSUM bank.
            # The warm-up burst covers the wait for the first x chunk; ballast
            # steps are woven into the pipeline at likely PE stall points so
            # the HAM clock gate keeps the PE at 2.4 GHz.
            wtile = cpool.tile([P, balw], BF16, name="wtile", tag="wtile")
            nc.vector.memset(wtile[:], 0.0)
            dps = dpool.tile([P, balw], F32, name="dps", tag="dps")

            def ballast():
                nc.tensor.matmul(
                    dps[:], wtile[:, 0:P], wtile[:], start=True, stop=True
                )

            for i in range(warmup):
                ballast()

            # software pipeline: L1(n) interleaved with mid(n-1) and dec(n-2)
            # so PE always has dense ready work through the tail
            for n in range(NCH + 2):
                a = emit_l1(n) if n < NCH else []
                b = emit_mid(n - 1) if 1 <= n <= NCH else []
                c = emit_dec(n - 2) if n >= 2 else []
                for step in ratio_merge(a, b, c):
                    step()

    nc.compile()
    _PROGRAM_CACHE[key] = nc
    return nc


# =====================================================================
# Host wrapper: shard, run, gather
# =====================================================================


def make_in_maps(
    x, enc_w0, enc_b0, enc_w1, enc_b1, enc_w2, enc_b2, qw,
    dec_w0, dec_b0, dec_w1, dec_b1, dec_w2, dec_b2,
):
    f32 = np.float32
    # W81 from the circuit, folded into the decoder's first layer
    w81 = _w81_from_qw(np.asarray(qw, dtype=np.float64))
    wfold = (w81 @ np.asarray(dec_w0, dtype=np.float64)).astype(f32)  # (81, 64)

    # enc_w0 repacked so SBUF col-block k holds rows k*128..(k+1)*128
    w0p = (
        np.asarray(enc_w0, f32).reshape(KD, P, H1).transpose(1, 0, 2).reshape(P, D)
    )
    w0p = np.ascontiguousarray(w0p)

    b2q = np.asarray(enc_b2, f32)[:NQ]
    pi2 = np.float32(np.pi / 2)
    bia = np.zeros((P, 21), dtype=f32)
    bia[:, 0] = enc_b0
    bia[:H2, 1] = enc_b1
    bia[0, 2] = pi2  # row 0 of feature stack: sin(pi/2) = 1
    bia[1 : 1 + NQ, 2] = b2q  # sin(t)
    bia[5 : 5 + NQ, 2] = b2q + pi2  # cos(t)
    bia[:H2, 3] = dec_b0
    bia[:H1, 4] = dec_b1
    bia[:, 5 : 5 + KD] = np.asarray(dec_b2, f32).reshape(KD, P).T

    w2q = np.asarray(enc_w2, f32)[:, :NQ]
    w2a = np.concatenate([np.zeros((H2, 1), f32), w2q, w2q], axis=1)

    bf16 = ml_dtypes.bfloat16
    common = {
        "w0p": w0p.astype(bf16),
        "w1": np.ascontiguousarray(np.asarray(enc_w1, f32)).astype(bf16),
        "w2a": np.ascontiguousarray(w2a).astype(bf16),
        "selc": _selection_matrices().astype(bf16),
        "wf": np.ascontiguousarray(wfold).astype(bf16),
        "wd1": np.ascontiguousarray(np.asarray(dec_w1, f32)).astype(bf16),
        "wd2": np.ascontiguousarray(np.asarray(dec_w2, f32)).astype(bf16),
        "bia": bia,
    }

    # pack x: core c, col XOFF[n] + k*cw + cc  <-  x[c*BL + C0S[n] + cc, k*128 + p]
    # host-cast to bf16 (same numerics as an in-DMA cast, half the HBM read)
    xr = np.asarray(x, f32).reshape(NCORES, BL, D)
    xp = np.empty((NCORES, P, XT), dtype=bf16)
    for n in range(NCH):
        c0, cw = C0S[n], CWS[n]
        seg = xr[:, c0 : c0 + cw, :].reshape(NCORES, cw, KD, P)
        xp[:, :, XOFF[n] : XOFF[n] + KD * cw] = (
            seg.transpose(0, 3, 2, 1).reshape(NCORES, P, KD * cw).astype(bf16)
        )

    in_maps = []
    for c in range(NCORES):
        m = dict(common)
        m["xpk"] = xp[c]
        in_maps.append(m)
    return in_maps


def gather_output(results):
    # outp[p, XOFF[n] + k*cw + c] = out[d = k*128 + p, b = C0S[n] + c]
    full = np.empty((D, B), dtype=np.float16)
    for c in range(NCORES):
        o = results[c]["outp"]
        for n in range(NCH):
            c0, cw = C0S[n], CWS[n]
            seg = o[:, XOFF[n] : XOFF[n] + KD * cw].reshape(P, KD, cw)
            full[:, c * BL + c0 : c * BL + c0 + cw] = (
                seg.transpose(1, 0, 2).reshape(D, cw)
            )
    return np.ascontiguousarray(full.T).astype(np.float32)  # (B, D)


def kernel(**inputs):
    nc = _build_program(
        dec2_bias_zero=not np.any(np.asarray(inputs["dec_b2"], np.float32))
    )
    in_maps = make_in_maps(**inputs)
    res = run_bass_kernel_spmd(nc, in_maps, core_ids=list(range(NCORES)))
    return gather_output(res.results)


if __name__ == "__main__":
    # quick self-exercise with random inputs (no reference available here)
    rng = np.random.default_rng(0)
    demo = {
        "x": rng.normal(size=(B, D)).astype(np.float32),
        "enc_w0": rng.normal(size=(D, H1)).astype(np.float32) * 0.02,
        "enc_b0": np.zeros(H1, np.float32),
        "enc_w1": rng.normal(size=(H1, H2)).astype(np.float32) * 0.02,
        "enc_b1": np.zeros(H2, np.float32),
        "enc_w2": rng.normal(size=(H2, L)).astype(np.float32) * 0.02,
        "enc_b2": np.zeros(L, np.float32),
        "qw": rng.normal(size=(NL, NQ, 3)).astype(np.float32),
        "dec_w0": rng.normal(size=(NQ, H2)).astype(np.float32) * 0.02,
        "dec_b0": np.zeros(H2, np.float32),
        "dec_w1": rng.normal(size=(H2, H1)).astype(np.float32) * 0.02,
        "dec_b1": np.zeros(H1, np.float32),
        "dec_w2": rng.normal(size=(H1, D)).astype(np.float32) * 0.02,
        "dec_b2": np.zeros(D, np.float32),
    }
    out = kernel(**demo)
    print("kernel ran, out shape:", out.shape, "finite:", np.isfinite(out).all())
